# revision 5
# baseline (speedup 1.0000x reference)
"""BottleneckAttention TRN2 kernel v3: 8 NeuronCores, one (batch, head) per core.

Decomposition (per core, batch b / head i):
  Scores via TWO fp8 DoubleRow matmuls per 128-key chunk (512-row contraction):
    pass1: KT=[k8;dk8 | Ih;Iw] x QA=[q8,q8 | rh8;rw8]
    pass2: KT (same)            x QB=[dq8,dq8 | drh8;drw8]
  where t8 = fp8e4(t), dt8 = fp8e4(t - t8) (residual pairs: ~0.13% storage
  error), Ih/Iw are 0/1 height/width indicator rows, rh/rw the rel-pos logits.
  This folds BOTH rel-biases additively into S and fully compensates fp8
  quantization -> no post-exp multiplies at all.
  exp paths per chunk (pattern below, pair-aligned):
    'a': ACT exact exp(S-6) -> fp8e4 e tile
    'd': DVE Schraudolph: uint8-saturate(A8*S + B8) bitcast fp8e5m2
         (saturation at 0 = exact underflow to +0.0)
  PV: all-fp8 DoubleRow pairs: o[0:65] += vt8[chunk-pair]^T e_pair, with a
  ones row for softmax denominators. Normalization: fast reciprocal of row 64,
  PE ones-outer broadcast, one TT mult into h_sb. Output projection partials
  DMA'd per quarter; host sums 4 per-head partials per batch + residual.
"""

import numpy as np
import ml_dtypes

import concourse.bass as bass
import concourse.bacc as bacc
import concourse.tile as tile
from concourse import mybir
from concourse.bass_utils import run_bass_kernel_spmd

F32 = mybir.dt.float32
BF16 = mybir.dt.bfloat16
FP8E4 = mybir.dt.float8e4
FP8E5 = mybir.dt.float8e5
U8 = mybir.dt.uint8
AF = mybir.ActivationFunctionType
DR = mybir.MatmulPerfMode.DoubleRow
SUB = mybir.AluOpType.subtract
MUL = mybir.AluOpType.mult
ADD = mybir.AluOpType.add

HEADS, B, C, HH, WW = 4, 2, 256, 64, 64
N = HH * WW           # 4096
DH = C // HEADS       # 64
NQ = 4                # query quarters
QB = N // NQ          # 1024
NJC = 32              # key chunks of 128
NPR = 16              # PV pairs
PVLAG = 7

A8 = 4.0 / np.log(2.0)          # e5m2 Schraudolph scale
SHIFT = 6.0                     # global exp shift (softmax-invariant)
B8 = 60.0 - 0.25 - A8 * SHIFT   # e5m2 bias - tuning const - shift

# per-quarter pair pattern: 'a' = ACT exact exp pair, 'd' = DVE schraudolph
PAIRS = ['a', 'd', 'a', 'd', 'a', 'd', 'a', 'd',
         'a', 'd', 'a', 'd', 'a', 'd', 'a', 'a']   # 9 a-pairs, 7 d-pairs


def _body(tc, io):
    from contextlib import ExitStack
    with ExitStack() as ctx:
        _body_inner(tc, io, ctx)


def _body_inner(tc, io, ctx):
    nc = tc.nc
    xb, wq, wk, wv, wo, relw, relh, ind, out = (
        io["xb"], io["wq"], io["wk"], io["wv"], io["wo"],
        io["relw"], io["relh"], io["ind"], io["out"],
    )

    big = ctx.enter_context(tc.tile_pool(name="big", bufs=1))
    ea_pool = ctx.enter_context(tc.tile_pool(name="ea", bufs=5))
    ed_pool = ctx.enter_context(tc.tile_pool(name="ed", bufs=4))
    ep = ctx.enter_context(tc.tile_pool(name="ep", bufs=2))
    spool = ctx.enter_context(tc.tile_pool(name="spool", bufs=2, space="PSUM"))
    opool = ctx.enter_context(tc.tile_pool(name="opool", bufs=1, space="PSUM"))
    ipool = ctx.enter_context(tc.tile_pool(name="ipool", bufs=1, space="PSUM"))

    # ---- SBUF tiles -------------------------------------------------
    xb_bf = big.tile([128, 2, N], BF16)
    wq_bf = big.tile([128, 2, DH], BF16)
    wk_bf = big.tile([128, 2, DH], BF16)
    wv_bf = big.tile([128, 2, DH], BF16)
    wo_bf = big.tile([64, 256], BF16)
    relw_bf = big.tile([64, 127], BF16)
    relh_bf = big.tile([64, 127], BF16)
    KT = big.tile([128, 2, N], FP8E4)    # kt0=[k8;dk8] built, kt1=[Ih;Iw] DMA
    QA = big.tile([128, 2, N], FP8E4)    # kt0=q8 dup, kt1=[rh8;rw8]
    QBt = big.tile([128, 2, N], FP8E4)   # kt0=dq8 dup, kt1=[drh8;drw8]
    vt8 = big.tile([128, NJC, 80], FP8E4)   # 80-row pitch: DR ldweights needs step%16==0
    h_sb = big.tile([64, N], BF16)
    bias_m6 = big.tile([128, 1], F32)
    ones64 = big.tile([1, 64], F32)

    # ---- input DMAs -------------------------------------------------
    xv = xb.rearrange("(cc p) n -> p cc n", p=128)
    nc.gpsimd.dma_start(out=wq_bf, in_=wq.rearrange("(cc p) d -> p cc d", p=128))
    nc.sync.dma_start(out=xb_bf[:, :, 0:QB], in_=xv[:, :, 0:QB])
    nc.scalar.dma_start(out=xb_bf[:, :, bass.ts(1, QB)], in_=xv[:, :, bass.ts(1, QB)])
    for t_bf, t_d in ((wk_bf, wk), (wv_bf, wv)):
        nc.gpsimd.dma_start(out=t_bf, in_=t_d.rearrange("(cc p) d -> p cc d", p=128))
    nc.sync.dma_start(out=xb_bf[:, :, bass.ts(2, QB)], in_=xv[:, :, bass.ts(2, QB)])
    nc.scalar.dma_start(out=xb_bf[:, :, bass.ts(3, QB)], in_=xv[:, :, bass.ts(3, QB)])
    nc.sync.dma_start(out=relh_bf, in_=relh)
    nc.scalar.dma_start(out=relw_bf, in_=relw)
    nc.sync.dma_start(out=KT[:, 1, :], in_=ind)
    nc.gpsimd.dma_start(out=wo_bf, in_=wo)

    nc.gpsimd.memset(bias_m6, -SHIFT)
    nc.vector.memset(ones64, 1.0)
    nc.gpsimd.memset(vt8[:, :, 64:80], 0.0)
    nc.gpsimd.memset(vt8[:, :, 64:65], 1.0)

    # PE warm-up while the first xb quarter lands.
    warm = big.tile([128, 512], BF16)
    nc.vector.memset(warm, 0.0)
    for _ in range(11):
        wps = spool.tile([128, 512], F32, tag="sp")
        nc.tensor.matmul(wps, warm[:, 0:128], warm, start=True, stop=True)

    # ---- build helpers ---------------------------------------------
    def build_q(qq, pool, tag):
        # q duplicated into both partition halves of psum; q8 copy + dq8 TT.
        ps = pool.tile([128, QB], F32, tag=tag)
        for half in range(2):
            for h in range(2):
                for cc in range(2):
                    nc.tensor.matmul(
                        ps[half * 64:(half + 1) * 64, bass.ts(h, 512)],
                        wq_bf[:, cc, :],
                        xb_bf[:, cc, qq * QB + h * 512: qq * QB + (h + 1) * 512],
                        start=(cc == 0), stop=(cc == 1))
        sl = bass.ts(qq, QB)
        nc.scalar.activation(out=QA[:, 0, sl], in_=ps, func=AF.Copy)
        nc.vector.tensor_tensor(out=QBt[:, 0, sl], in0=ps, in1=QA[:, 0, sl], op=SUB)

    def build_k(g, pool, tag):
        ps = pool.tile([128, QB], F32, tag=tag)
        for half in range(2):
            for h in range(2):
                for cc in range(2):
                    nc.tensor.matmul(
                        ps[half * 64:(half + 1) * 64, bass.ts(h, 512)],
                        wk_bf[:, cc, :],
                        xb_bf[:, cc, g * QB + h * 512: g * QB + (h + 1) * 512],
                        start=(cc == 0), stop=(cc == 1))
        sl = bass.ts(g, QB)
        nc.scalar.activation(out=KT[0:64, 0, sl], in_=ps[0:64, :], func=AF.Copy)
        nc.vector.tensor_tensor(out=KT[64:128, 0, sl], in0=ps[64:128, :],
                                in1=KT[0:64, 0, sl], op=SUB)

    def build_rh(g, pool, tag):
        # RH^T[jh, n=(x,y)] = sum_d relh[jh - x + 63, d] * q8[d, n]
        ps = pool.tile([128, QB], F32, tag=tag)
        for xi in range(16):
            xx = g * 16 + xi
            nc.tensor.matmul(
                ps[0:64, bass.ts(xi, 64)],
                relh_bf[:, 63 - xx: 127 - xx],
                QA[0:64, 0, xx * 64: (xx + 1) * 64],
                start=True, stop=True)
        sl = bass.ts(g, QB)
        nc.scalar.activation(out=QA[0:64, 1, sl], in_=ps[0:64, :], func=AF.Copy)
        nc.vector.tensor_tensor(out=QBt[0:64, 1, sl], in0=ps[0:64, :],
                                in1=QA[0:64, 1, sl], op=SUB)

    q_xy = QA[0:64, 0, :].rearrange("d (x y) -> d x y", y=64)
    rw_a_xy = QA[64:128, 1, :].rearrange("jw (x y) -> jw x y", y=64)
    rw_b_xy = QBt[64:128, 1, :].rearrange("jw (x y) -> jw x y", y=64)

    def build_rw(g, pool, tag):
        # RW^T[jw, n=(x,y)] = sum_d relw[jw - y + 63, d] * q8[d, n]; y-block g.
        ps = pool.tile([128, QB], F32, tag=tag)
        for yi in range(16):
            yy = g * 16 + yi
            nc.tensor.matmul(
                ps[0:64, bass.ts(yi, 64)],
                relw_bf[:, 63 - yy: 127 - yy],
                q_xy[:, :, yy],
                start=True, stop=True)
        ps_t = ps[0:64, :].rearrange("p (yi x) -> p x yi", x=64)
        osl = slice(g * 16, (g + 1) * 16)
        nc.scalar.activation(out=rw_a_xy[:, :, osl], in_=ps_t, func=AF.Copy)
        nc.vector.tensor_tensor(out=rw_b_xy[:, :, osl], in0=ps_t,
                                in1=rw_a_xy[:, :, osl], op=SUB)

    def build_v(g, pool, tag):
        ps = pool.tile([128, 8, 64], F32, tag=tag)
        for ci in range(8):
            chunk = g * 8 + ci
            for cc in range(2):
                nc.tensor.matmul(
                    ps[:, ci, :],
                    xb_bf[:, cc, chunk * 128: (chunk + 1) * 128],
                    wv_bf[:, cc, :],
                    start=(cc == 0), stop=(cc == 1))
        nc.scalar.activation(out=vt8[:, g * 8:(g + 1) * 8, 0:64], in_=ps, func=AF.Copy)

    # ---- prologue ---------------------------------------------------
    slots = [(spool, "sp"), (spool, "sp"), (ipool, "ij"), (opool, "ov")]
    builds = [("k", 0), ("q", 0), ("k", 1), ("q", 1), ("k", 2), ("q", 2),
              ("k", 3), ("q", 3), ("rh", 0), ("rw", 0), ("rw", 1), ("rw", 2),
              ("rw", 3), ("v", 0), ("v", 1)]
    fn = {"k": build_k, "q": build_q, "rh": build_rh, "rw": build_rw, "v": build_v}
    for i, (kind, idx) in enumerate(builds):
        pool, tag = slots[i % 4]
        fn[kind](idx, pool, tag)

    # ---- main loop helpers -----------------------------------------
    chunk_info = []   # (pair, pos, kind) per chunk
    for pr, kind in enumerate(PAIRS):
        chunk_info.append((pr, 0, kind))
        chunk_info.append((pr, 1, kind))

    o_ps = None
    e_tiles = [None] * NPR

    def s_stage(qq, jc):
        sp = spool.tile([128, QB], F32, tag="sp")
        kslice = KT[:, :, jc * 128: (jc + 1) * 128]
        for h in range(2):
            r0 = qq * QB + h * 512
            nc.tensor.matmul(sp[:, bass.ts(h, 512)], kslice, QA[:, :, r0:r0 + 512],
                             start=True, stop=False, perf_mode=DR)
            nc.tensor.matmul(sp[:, bass.ts(h, 512)], kslice, QBt[:, :, r0:r0 + 512],
                             start=False, stop=True, perf_mode=DR)
        pr, pos, kind = chunk_info[jc]
        if kind == 'a':
            if pos == 0:
                e_tiles[pr] = ea_pool.tile([128, 2, QB], FP8E4, tag="ea", name="ea_t")
            nc.scalar.activation(out=e_tiles[pr][:, pos, :], in_=sp,
                                 func=AF.Exp, bias=bias_m6)
        else:
            if pos == 0:
                e_tiles[pr] = ed_pool.tile([128, 2, QB], FP8E5, tag="ed", name="ed_t")
            nc.vector.tensor_scalar(out=e_tiles[pr][:, pos, :].bitcast(U8),
                                    in0=sp, scalar1=A8, scalar2=B8,
                                    op0=MUL, op1=ADD)

    def pv_stage(pr):
        e = e_tiles[pr]
        for h in range(2):
            nc.tensor.matmul(o_ps[0:80, bass.ts(h, 512)],
                             vt8[:, 2 * pr: 2 * pr + 2, :],
                             e[:, :, bass.ts(h, 512)],
                             start=(pr == 0), stop=(pr == NPR - 1), perf_mode=DR)
        e_tiles[pr] = None

    def drain(qqp, pool, tag):
        # normalize previous quarter's o_ps into h_sb (frees o_ps for reuse)
        rs0 = ep.tile([1, QB], F32, tag="rs0")
        nc.scalar.activation(out=rs0, in_=o_ps[64:65, :], func=AF.Copy)
        rsb = ep.tile([1, QB], F32, tag="rsb")
        nc.vector.reciprocal_approx_fast(out=rsb, in_=rs0)
        rbc_ps = pool.tile([128, QB], F32, tag=tag)
        for h in range(2):
            nc.tensor.matmul(rbc_ps[0:64, bass.ts(h, 512)], ones64,
                             rsb[:, bass.ts(h, 512)], start=True, stop=True)
        rbc = ep.tile([64, QB], BF16, tag="rbc")
        nc.scalar.activation(out=rbc, in_=rbc_ps[0:64, :], func=AF.Copy)
        nc.vector.tensor_tensor(out=h_sb[:, bass.ts(qqp, QB)],
                                in0=o_ps[0:64, :], in1=rbc, op=MUL)

    def proj(qqp, oh, pool, tag, ceng):
        pp = pool.tile([128, QB], F32, tag=tag)
        for h in range(2):
            nc.tensor.matmul(pp[:, bass.ts(h, 512)],
                             wo_bf[:, oh * 128: (oh + 1) * 128],
                             h_sb[:, qqp * QB + h * 512: qqp * QB + (h + 1) * 512],
                             start=True, stop=True)
        osb = ep.tile([128, QB], BF16, tag="osb")
        if ceng == 'act':
            nc.scalar.activation(out=osb, in_=pp, func=AF.Copy)
        else:
            nc.vector.tensor_copy(out=osb, in_=pp)
        eng = nc.sync if oh == 0 else nc.gpsimd
        eng.dma_start(out=out[oh * 128: (oh + 1) * 128, qqp * QB: (qqp + 1) * QB],
                      in_=osb)

    # ---- main loop --------------------------------------------------
    for qq in range(NQ):
        if qq > 0:
            drain(qq - 1, ipool, "ij")
        o_ps = opool.tile([128, QB], F32, tag="ov")
        for t in range(NJC + PVLAG + 1):
            if t < NJC:
                s_stage(qq, t)
            if qq == 0:
                if t == 2:
                    build_v(2, ipool, "ij")
                elif t == 6:
                    build_v(3, ipool, "ij")
            if qq < NQ - 1 and t == 10:
                build_rh(qq + 1, ipool, "ij")
            if qq > 0:
                if t == 16:
                    proj(qq - 1, 0, ipool, "ij", 'act')
                elif t == 20:
                    proj(qq - 1, 1, ipool, "ij", 'dve')
            if t > PVLAG and (t - PVLAG) % 2 == 0:
                pv_stage((t - PVLAG) // 2 - 1)

    # tail: final quarter drain + projections on free spool slots
    drain(NQ - 1, spool, "sp")
    proj(NQ - 1, 0, spool, "sp", 'act')
    proj(NQ - 1, 1, ipool, "ij", 'dve')


_NC_CACHE = {}


def _build():
    if "nc" in _NC_CACHE:
        return _NC_CACHE["nc"]
    nc = bacc.Bacc("TRN2", target_bir_lowering=False, debug=False, num_devices=8)
    io = {
        "xb": nc.dram_tensor("xb", [C, N], BF16, kind="ExternalInput").ap(),
        "wq": nc.dram_tensor("wq", [C, DH], BF16, kind="ExternalInput").ap(),
        "wk": nc.dram_tensor("wk", [C, DH], BF16, kind="ExternalInput").ap(),
        "wv": nc.dram_tensor("wv", [C, DH], BF16, kind="ExternalInput").ap(),
        "wo": nc.dram_tensor("wo", [DH, C], BF16, kind="ExternalInput").ap(),
        "relw": nc.dram_tensor("relw", [DH, 127], BF16, kind="ExternalInput").ap(),
        "relh": nc.dram_tensor("relh", [DH, 127], BF16, kind="ExternalInput").ap(),
        "ind": nc.dram_tensor("ind", [128, N], FP8E4, kind="ExternalInput").ap(),
        "out": nc.dram_tensor("out", [C, N], BF16, kind="ExternalOutput").ap(),
    }
    with tile.TileContext(nc) as tc:
        _body(tc, io)
    nc.compile()
    _NC_CACHE["nc"] = nc
    return nc


_last_in_maps = None


def kernel(x, w_qkv, w_out, rel_height, rel_width):
    global _last_in_maps
    bf16 = ml_dtypes.bfloat16
    fp8 = ml_dtypes.float8_e4m3
    x = np.ascontiguousarray(np.asarray(x, np.float32))
    w_qkv = np.asarray(w_qkv, np.float32)
    w_out = np.asarray(w_out, np.float32)
    rel_height = np.asarray(rel_height, np.float32)
    rel_width = np.asarray(rel_width, np.float32)

    scale = np.float32(DH ** -0.5)
    jj = np.arange(N)
    ind_np = np.zeros((128, N), np.float32)
    ind_np[0:64] = (jj[None, :] // WW == np.arange(64)[:, None])
    ind_np[64:128] = (jj[None, :] % WW == np.arange(64)[:, None])
    ind_const = np.ascontiguousarray(ind_np.astype(fp8))
    relw_t = np.ascontiguousarray(rel_width.T.astype(bf16))
    relh_t = np.ascontiguousarray(rel_height.T.astype(bf16))

    xb_bf = [np.ascontiguousarray(x[b].reshape(C, N).astype(bf16)) for b in range(B)]

    in_maps = []
    for g in range(8):
        b, i = divmod(g, HEADS)
        sl = slice(i * DH, (i + 1) * DH)
        in_maps.append({
            "xb": xb_bf[b],
            "wq": np.ascontiguousarray((w_qkv[i * DH:(i + 1) * DH] * scale).T.astype(bf16)),
            "wk": np.ascontiguousarray(w_qkv[C + i * DH: C + (i + 1) * DH].T.astype(bf16)),
            "wv": np.ascontiguousarray(w_qkv[2 * C + i * DH: 2 * C + (i + 1) * DH].T.astype(bf16)),
            "wo": np.ascontiguousarray(w_out[:, sl].T.astype(bf16)),
            "relw": relw_t,
            "relh": relh_t,
            "ind": ind_const,
        })

    _last_in_maps = in_maps
    nc = _build()
    res = run_bass_kernel_spmd(nc, in_maps, core_ids=list(range(8)))
    parts = [np.asarray(r["out"]).astype(np.float32) for r in res.results]
    outf = np.empty((B, C, N), np.float32)
    for b in range(B):
        outf[b] = parts[4 * b] + parts[4 * b + 1] + parts[4 * b + 2] + parts[4 * b + 3]
        outf[b] += x[b].reshape(C, N)
    return outf.reshape(B, C, HH, WW)
